# revision 24
# baseline (speedup 1.0000x reference)
"""Trainium2 Bass kernel for nn_MultiHeadEFRALayer (sparse_attention).

Strategy: shard the 16384 genes across 8 NeuronCores (2048 each); every core
computes all 8 heads x all 512 TFs for its gene slice.  The 3x3 evidence gate
softmax is reformulated in terms of pairwise score differences so the whole
per-element chain runs in fp16 on VectorE/ScalarE, the per-head scale /
sqrt(dk) and gate-mixing coefficients are folded into the Q-projection
weights on the host, and all means over heads are computed on the TensorE
via selector matmuls (heads are packed into the partition dimension).
Softmax over genes needs the global per-(head,TF) sum of exp(u): a tiny
[128,32] AllReduce mid-kernel provides it; a second pass over the archived
exp(u) tiles (DRAM) produces A_mean.
"""

import sys
sys.path.insert(0, "/opt/trn_rl_repo")

import math
import numpy as np

import concourse.bass as bass
import concourse.mybir as mybir
import concourse.tile as tile
from concourse import bacc
from concourse.bass_utils import run_bass_kernel_spmd

N_CORES = 8
H, DK, D, T, G = 8, 64, 512, 512, 16384
GS = G // N_CORES          # genes per core
KG = 6.0                   # gate-exp shift (overflow guard)
KE = 4.0                   # gene-softmax exp shift
E6 = float(math.exp(KG))
EMK = float(math.exp(-KG))
F32 = mybir.dt.float32
F16 = mybir.dt.float16
AF = mybir.ActivationFunctionType
OP = mybir.AluOpType

_BUILD_CACHE = {}
_ACT_PATCHED = False


def _patch_act_tables():
    """Make Bacc's activation-table chooser pick the combined
    natural_log_exp_and_others set for both Exp and Ln (avoids a ~1.3us
    table reload between every Ln and Exp). We present it a table map with
    Exp/Ln stripped from the exp-only / ln-only sets (dict order, and hence
    act_func_set_id numbering, is unchanged)."""
    global _ACT_PATCHED
    if _ACT_PATCHED:
        return
    try:
        from concourse import bacc as _bacc_mod
        from concourse.hw_specs import get_activation_tables as _orig
    except Exception:
        _ACT_PATCHED = True
        return

    def patched(arch):
        tabs = _orig(arch)
        out = {}
        for name, funcs in tabs.items():
            f = set(funcs)
            if name == "exp_and_others":
                f.discard(AF.Exp)
            if name == "natural_log":
                f.discard(AF.Ln)
            out[name] = f
        return out

    _bacc_mod.get_activation_tables = patched
    _ACT_PATCHED = True


def _unpatch_guard():
    pass
LAST_RESULT = None  # BassKernelResults of the most recent run (for test.py)


def _build(identity_gate: bool):
    """Build the Bacc program. Returns nc."""
    if identity_gate:
        NM, NS = 5, 3
        # bank layout in score PSUM: 0=x, 1=y, 2=s0 ; T12 reads banks 0:2
        combos = {0: [(0, 1), (1, 0)], 1: [(2, 2), (3, 0)], 2: [(4, 0)]}
        XS = 0
    else:
        NM, NS = 11, 5
        # banks: 0=x_l, 1=y_l, 2=x_s, 3=y_s, 4=s0
        combos = {0: [(0, 0), (1, 1), (2, 2)], 1: [(3, 0), (4, 1), (5, 2)],
                  2: [(6, 1), (7, 0)], 3: [(8, 2), (9, 0)], 4: [(10, 0)]}
        XS = 2
    S0B = NS - 1  # s0 bank index

    _patch_act_tables()
    nc = bacc.Bacc("TRN2", num_devices=N_CORES)

    # ---------------- I/O ----------------
    zt_dr = nc.dram_tensor("zt", (3, 128, 4, GS), F16, kind="ExternalInput")
    ztf_dr = nc.dram_tensor("ztf", (3, 128, 4, T), F16, kind="ExternalInput")
    wk_dr = nc.dram_tensor("wk", (3, 128, 4, D), F16, kind="ExternalInput")
    wq_dr = nc.dram_tensor("wq", (NM, 128, 4, D), F16, kind="ExternalInput")
    bx_dr = nc.dram_tensor("bx", (2, 128, 1), F32, kind="ExternalInput")
    by_dr = nc.dram_tensor("by", (2, 128, 1), F32, kind="ExternalInput")
    um_dr = nc.dram_tensor("u_mean", (T, GS), F16, kind="ExternalOutput")
    a0m_dr = nc.dram_tensor("a0_mean", (T, GS), F16, kind="ExternalOutput")
    a1m_dr = nc.dram_tensor("a1_mean", (T, GS), F16, kind="ExternalOutput")
    Am_dr = nc.dram_tensor("A_mean", (T, GS), F16, kind="ExternalOutput")

    e_arch = nc.dram_tensor("e_arch", (32, 128, GS), F16)
    zc_in = [nc.dram_tensor(f"zc_in{h}", (128, 16), F32) for h in range(2)]
    zc_out = [nc.dram_tensor(f"zc_out{h}", (128, 16), F32, addr_space="Shared")
              for h in range(2)]

    # selector constants: partitions p = h_local*32 + t_local
    pp_, cc_ = np.arange(128)[:, None], np.arange(32)[None, :]
    mask01 = (pp_ % 32 == cc_).astype(np.float16)
    sel8_np = (mask01 / 8.0).astype(np.float16)
    sela0_np = (mask01 * (float(np.exp(-KG)) / 8.0)).astype(np.float16)
    mask_dr = nc.inline_tensor(mask01, "mask01")
    cbias_np = np.tile(np.array([[-KG, EMK, -KE, 0.0]], np.float32), (128, 1))
    cbias_dr = nc.inline_tensor(cbias_np, "cbias")
    sel8_dr = nc.inline_tensor(sel8_np, "sel8")
    sela0_dr = nc.inline_tensor(sela0_np, "sela0")

    with tile.TileContext(nc) as tc:
        with tc.tile_pool(name="persist", bufs=1) as pp, \
             tc.tile_pool(name="psum_s", bufs=(2 if identity_gate else 1), space="PSUM") as ps_s, \
             tc.tile_pool(name="psum_m", bufs=1, space="PSUM") as ps_m:
            cp_ctx = tc.tile_pool(name="chain", bufs=(2 if identity_gate else 1))

            # ---------------- constants / small tiles ----------------
            mask_t = pp.tile([128, 32], F16, tag="mask01")
            sel8_t = pp.tile([128, 32], F16, tag="sel8")
            sela0_t = pp.tile([128, 32], F16, tag="sela0")
            cbias_t = pp.tile([128, 4], F32, tag="cbias")
            nc.sync.dma_start(out=cbias_t[:], in_=cbias_dr[:])
            b_mKG = cbias_t[:, 0:1]
            b_EMK = cbias_t[:, 1:2]
            b_mKE = cbias_t[:, 2:3]
            b_zero = cbias_t[:, 3:4]
            nc.sync.dma_start(out=mask_t[:], in_=mask_dr[:])
            nc.sync.dma_start(out=sel8_t[:], in_=sel8_dr[:])
            nc.sync.dma_start(out=sela0_t[:], in_=sela0_dr[:])
            z_sb = pp.tile([128, 32], F32, tag="z_sb")
            bxq = [pp.tile([128, 1], F32, tag=f"bx{i}", name=f"bx{i}") for i in range(2)]
            byq = [pp.tile([128, 1], F32, tag=f"by{i}", name=f"by{i}") for i in range(2)]
            for i in range(2):
                nc.sync.dma_start(out=bxq[i][:], in_=bx_dr[i])
                nc.sync.dma_start(out=byq[i][:], in_=by_dr[i])

            # ---------------- prologue: load weights, project Q and K ----
            qcT = [pp.tile([128, 4, T], F16, tag=f"qc{m}", name=f"qc{m}") for m in range(NM)]
            kT = [[pp.tile([128, GS], F16, tag=f"kT{e}_{c}", name=f"kT{e}_{c}") for c in range(4)]
                  for e in range(3)]
            PRO_W = 1024 if identity_gate else 512
            NPW = PRO_W // 512

            with tc.tile_pool(name="pro_in", bufs=1) as prop:
                ztf_t = []
                wq_t = []
                for e in range(3):
                    t_ = prop.tile([128, 4, T], F16, tag=f"ztf{e}", name=f"ztf{e}")
                    nc.sync.dma_start(out=t_[:], in_=ztf_dr[e])
                    ztf_t.append(t_)
                for m in range(NM):
                    t_ = prop.tile([128, 4, D], F16, tag="wq", bufs=3, name=f"wq{m}")
                    nc.sync.dma_start(out=t_[:], in_=wq_dr[m])
                    wq_t.append(t_)

                # channel of each folded weight m
                ch_of_m = {m: e for k in combos for (m, e) in combos[k]}

                # Q projections: qcT[m][:, ch, :] = sum_c wq_m[c]^T-block @ ztf
                for m in range(NM):
                    e = ch_of_m[m]
                    for blk in range(4 // NPW):
                        qps = ps_m.tile([128, NPW, 512], F32, tag="mean", name="qps")
                        for cc in range(NPW):
                            ch = blk * NPW + cc
                            for c in range(4):
                                nc.tensor.matmul(
                                    out=qps[:, cc, :],
                                    lhsT=wq_t[m][:, c, ch * 128:(ch + 1) * 128],
                                    rhs=ztf_t[e][:, c, :],
                                    start=(c == 0), stop=(c == 3))
                        if m % 2 == 0:
                            nc.scalar.copy(out=qcT[m][:, blk * NPW:(blk + 1) * NPW, :],
                                           in_=qps[:])
                        else:
                            nc.vector.tensor_copy(out=qcT[m][:, blk * NPW:(blk + 1) * NPW, :],
                                                  in_=qps[:])

                wk_t = []
                zt_t = []
                for e in range(3):
                    t_ = prop.tile([128, 4, D], F16, tag=f"wk{e}", name=f"wk{e}")
                    nc.sync.dma_start(out=t_[:], in_=wk_dr[e])
                    wk_t.append(t_)
                    t_ = prop.tile([128, 4, GS], F16, tag=f"zt{e}", name=f"zt{e}")
                    nc.sync.dma_start(out=t_[:], in_=zt_dr[e])
                    zt_t.append(t_)

                # K projections: kT[e][ch] = sum_c wk[e][c]^T-block @ zt[e][c]
                # ch-major so the first supertiles' inputs finish first
                for ch in range(4):
                    for e in range(3):
                        for blk in range(GS // 512 // NPW):
                            kps = ps_m.tile([128, NPW, 512], F32, tag="mean", name="kps")
                            for gg in range(NPW):
                                g0 = blk * PRO_W + gg * 512
                                for c in range(4):
                                    nc.tensor.matmul(
                                        out=kps[:, gg, :],
                                        lhsT=wk_t[e][:, c, ch * 128:(ch + 1) * 128],
                                        rhs=zt_t[e][:, c, g0:g0 + 512],
                                        start=(c == 0), stop=(c == 3))
                            if (ch + blk) % 2 == 0:
                                nc.scalar.copy(
                                    out=kT[e][ch][:, blk * PRO_W:(blk + 1) * PRO_W],
                                    in_=kps[:])
                            else:
                                nc.vector.tensor_copy(
                                    out=kT[e][ch][:, blk * PRO_W:(blk + 1) * PRO_W],
                                    in_=kps[:])

            cp = cp_ctx.__enter__()
            # ---------------- main loop ----------------
            zg = pp.tile([128, 32], F32, tag="zg")
            rz = pp.tile([128, 32], F32, tag="rz")

            def z_allreduce(hf):
                cs = slice(hf * 16, (hf + 1) * 16)
                nc.sync.dma_start(out=zc_in[hf][:], in_=z_sb[:, cs])
                nc.gpsimd.collective_compute(
                    "AllReduce", OP.add,
                    replica_groups=[[r for r in range(N_CORES)]],
                    ins=[zc_in[hf][:]], outs=[zc_out[hf][:]])
                nc.sync.dma_start(out=zg[:, cs], in_=zc_out[hf][:])
                nc.vector.reciprocal(out=rz[:, cs], in_=zg[:, cs])
                # rz := 2^14 / (8 * Z)
                nc.vector.tensor_scalar_mul(rz[:, cs], rz[:, cs],
                                            float(2.0 ** 14 / 8.0))

            ut_t = [None, None]
            a0_t = [None, None]
            v1_t = [None, None]
            for j in range(16):            # t-block of 32 TFs
                for i in range(2):         # head quad (heads 4i..4i+3)
                    st = i * 16 + j
                    zcol = j * 2 + i
                    e_cat = cp.tile([128, 2, GS], F16, tag="ecat", name=f"ecat{i}")
                    c0 = cp.tile([128, GS], F16, tag="c0", name=f"c0{i}")
                    t12 = cp.tile([128, 2, GS], F16, tag="t12", name=f"t12{i}")
                    for q in range(4):
                        qs = slice(q * 512, (q + 1) * 512)
                        sxy = ps_s.tile([128, NS, 512], F32, tag="s", name="sxy")
                        # ---- score matmuls ----
                        # Emission interleaves banks so that one bank's next
                        # col-group start (bank-bit clear) never races the
                        # previous group's accumulate drain.
                        max_p = max(len(v) for v in combos.values())
                        for pi in range(max_p):
                            for k in range(NS):
                                plist = combos[k]
                                if pi >= len(plist):
                                    continue
                                m, e = plist[pi]
                                for hm in range(4):
                                    head = 4 * i + hm
                                    ch = head // 2
                                    po = (head % 2) * 64
                                    nc.tensor.matmul(
                                        out=sxy[hm * 32:(hm + 1) * 32, k, :],
                                        lhsT=qcT[m][po:po + 64, ch,
                                                    j * 32:(j + 1) * 32],
                                        rhs=kT[e][ch][po:po + 64, qs],
                                        start=(pi == 0),
                                        stop=(pi == len(plist) - 1),
                                        tile_position=(po, hm * 32),
                                        skip_group_check=True)
                        # ---- chain entry (per q) ----
                        if identity_gate:
                            nc.scalar.activation(out=e_cat[:, :, qs],
                                                 in_=sxy[:, 0:2, :],
                                                 func=AF.Exp, bias=b_mKG)
                        else:
                            nc.scalar.activation(out=e_cat[:, 0, qs],
                                                 in_=sxy[:, 0, :],
                                                 func=AF.Exp, bias=bxq[i][:])
                            nc.scalar.activation(out=e_cat[:, 1, qs],
                                                 in_=sxy[:, 1, :],
                                                 func=AF.Exp, bias=byq[i][:])
                        nc.vector.tensor_tensor(out=t12[:, :, qs],
                                                in0=sxy[:, XS:XS + 2, :],
                                                in1=e_cat[:, :, qs], op=OP.mult)
                        if q < 2:
                            nc.scalar.copy(out=c0[:, qs], in_=sxy[:, S0B, :])
                        else:
                            nc.vector.tensor_copy(out=c0[:, qs],
                                                  in_=sxy[:, S0B, :])
                    # ---- chain mid (full 2048) ----
                    tsum = cp.tile([128, GS], F16, tag="tsum", name=f"tsum{i}")
                    nc.vector.tensor_add(out=tsum[:], in0=e_cat[:, 0, :],
                                         in1=e_cat[:, 1, :])
                    Lt = cp.tile([128, GS], F16, tag="L", name=f"L{i}")
                    nc.scalar.activation(out=Lt[:], in_=tsum[:], func=AF.Ln,
                                         bias=b_EMK)
                    a0 = cp.tile([128, GS], F16, tag=f"a0{i}", name=f"a0{i}", bufs=1)
                    nc.scalar.activation(out=a0[:], in_=Lt[:], func=AF.Exp,
                                         scale=-1.0, bias=b_zero)
                    Pp = cp.tile([128, GS], F16, tag="Pp", name=f"Pp{i}")
                    nc.vector.tensor_add(out=Pp[:], in0=t12[:, 0, :],
                                         in1=t12[:, 1, :])
                    Wt = cp.tile([128, GS], F16, tag="Wt", name=f"Wt{i}", bufs=1)
                    nc.vector.tensor_mul(out=Wt[:], in0=Pp[:], in1=a0[:])
                    ut = cp.tile([128, GS], F16, tag=f"ut{i}", name=f"ut{i}", bufs=1)
                    nc.vector.tensor_add(out=ut[:], in0=Wt[:], in1=c0[:])
                    Et = cp.tile([128, GS], F16, tag="Et", name=f"Et{i}")
                    nc.scalar.activation(out=Et[:], in_=ut[:], func=AF.Exp,
                                         bias=b_mKE, accum_out=z_sb[:, zcol:zcol + 1])
                    v1 = cp.tile([128, GS], F16, tag=f"v1{i}", name=f"v1{i}", bufs=1)
                    nc.vector.tensor_mul(out=v1[:], in0=e_cat[:, 0, :],
                                         in1=a0[:])
                    nc.sync.dma_start(out=e_arch[st], in_=Et[:])
                    ut_t[i], a0_t[i], v1_t[i] = ut, a0, v1

                # ---- head-mean matmuls (one PSUM tile per quad; the two
                # quads are combined by the fused DVE add so the shared mean
                # slot is released quickly between chains) ----
                stgF = cp.tile([128, 2, 2, 512], F16, tag="mstageF",
                               name="stgF")
                for half in range(2):
                    stg0 = cp.tile([128, 2, 512], F32, tag="mstage0", name="stg0")
                    for i in range(2):
                        mps = ps_m.tile([128, 2, 512], F32, tag="mean",
                                        name="mps")
                        for gg in range(2):
                            g0 = half * 1024 + gg * 512
                            for bp, rhs_t, sel_t in (
                                    (0, ut_t[i], sel8_t),
                                    (32, a0_t[i], sela0_t),
                                    (64, v1_t[i], sel8_t)):
                                nc.tensor.matmul(
                                    out=mps[bp:bp + 32, gg, :],
                                    lhsT=sel_t[:],
                                    rhs=rhs_t[:, g0:g0 + 512],
                                    start=True, stop=True,
                                    tile_position=(0, bp),
                                    skip_group_check=True)
                        if i == 0:
                            nc.scalar.copy(out=stg0[0:96, :, :],
                                           in_=mps[0:96, :, :])
                        else:
                            nc.vector.scalar_tensor_tensor(
                                out=stgF[0:96, half, :, :], in0=mps[0:96, :, :],
                                scalar=1.0, in1=stg0[0:96, :, :],
                                op0=OP.mult, op1=OP.add)
                rs = slice(j * 32, (j + 1) * 32)
                nc.sync.dma_start(out=um_dr[rs, :], in_=stgF[0:32])
                nc.sync.dma_start(out=a0m_dr[rs, :], in_=stgF[32:64])
                nc.sync.dma_start(out=a1m_dr[rs, :], in_=stgF[64:96])
                if j == 7:
                    z_allreduce(0)
                elif j == 15:
                    z_allreduce(1)

            # ---------------- pass 2 (A_mean) ----------------
            for j in range(16):
                Er = [None, None]
                selr = [None, None]
                for i in range(2):
                    st = i * 16 + j
                    zcol = j * 2 + i
                    Er[i] = cp.tile([128, GS], F16, tag=f"Ere{i}", name=f"Ere{i}", bufs=1)
                    nc.sync.dma_start(out=Er[i][:], in_=e_arch[st])
                    selr[i] = cp.tile([128, 32], F16, tag=f"selr{i}", name=f"selr{i}")
                    nc.vector.tensor_scalar(out=selr[i][:], in0=mask_t[:],
                                            scalar1=rz[:, zcol:zcol + 1],
                                            scalar2=None, op0=OP.mult)
                astgF = cp.tile([128, 2, 2, 512], F16, tag="astageF",
                                name="astgF", bufs=1)
                for half in range(2):
                    astg0 = cp.tile([128, 2, 512], F32, tag="astage0",
                                    name="astg0", bufs=1)
                    for i in range(2):
                        aps = ps_m.tile([128, 2, 512], F32, tag="mean",
                                        name="aps")
                        for gg in range(2):
                            g0 = half * 1024 + gg * 512
                            nc.tensor.matmul(
                                out=aps[0:32, gg, :], lhsT=selr[i][:],
                                rhs=Er[i][:, g0:g0 + 512],
                                start=True, stop=True,
                                tile_position=(0, 0),
                                skip_group_check=True)
                        if i == 0:
                            nc.scalar.mul(out=astg0[0:32, :, :],
                                          in_=aps[0:32, :, :],
                                          mul=float(2.0 ** -14))
                        else:
                            nc.vector.scalar_tensor_tensor(
                                out=astgF[0:32, half, :, :], in0=aps[0:32, :, :],
                                scalar=float(2.0 ** -14), in1=astg0[0:32, :, :],
                                op0=OP.mult, op1=OP.add)
                rs = slice(j * 32, (j + 1) * 32)
                nc.sync.dma_start(out=Am_dr[rs, :], in_=astgF[0:32])
            cp_ctx.__exit__(None, None, None)

    nc.compile()
    return nc


def _prep_inputs(inputs):
    """Host-side prep: gather, transpose, fold, fp16 pack. Returns
    (identity_gate, common in_map pieces, per-core zt slices)."""
    f16 = np.float16
    tf_idx = np.asarray(inputs["tf_idx"])
    Wg = np.asarray(inputs["gate_weight"], dtype=np.float32)
    bg = np.asarray(inputs["gate_bias"], dtype=np.float32)
    zs = [np.asarray(inputs["z_seq"], np.float32),
          np.asarray(inputs["z_exp"], np.float32),
          np.asarray(inputs["z_txt"], np.float32)]
    Wqs = [np.asarray(inputs["Wq_seq"], np.float32),
           np.asarray(inputs["Wq_exp"], np.float32),
           np.asarray(inputs["Wq_txt"], np.float32)]
    Wks = [np.asarray(inputs["Wk_seq"], np.float32),
           np.asarray(inputs["Wk_exp"], np.float32),
           np.asarray(inputs["Wk_txt"], np.float32)]

    eye = np.tile(np.eye(3, dtype=np.float32)[None], (H, 1, 1))
    identity_gate = bool(np.allclose(Wg, eye) and np.allclose(bg, 0.0))

    def chunked(mat, width):
        # [D, width] -> [128, 4, width] with [p, c, :] = mat[c*128+p, :]
        return np.ascontiguousarray(
            mat.reshape(4, 128, width).transpose(1, 0, 2)).astype(f16)

    # z^T  [D, G] -> zt[e]: [128, 4, G] (sliced per core later)
    zt_full = np.stack([chunked(z.T, G) for z in zs])          # [3,128,4,G]
    ztf = np.stack([chunked(z[tf_idx].T, T) for z in zs])      # [3,128,4,T]
    wk = np.stack([chunked(w, D) for w in Wks])                # [3,128,4,D]

    def fold(e, coef_per_head):
        # Wq_e scaled per head by coef/8 (score scale 1/sqrt(dk)=1/8)
        return Wqs[e] * np.repeat(coef_per_head / 8.0, DK)[None, :]

    ones = np.ones(H, np.float32)
    if identity_gate:
        folded = [fold(1, ones), fold(0, -ones),
                  fold(2, ones), fold(0, -ones),
                  fold(0, ones)]
        bx = np.zeros(H, np.float32)
        by = np.zeros(H, np.float32)
    else:
        gx = Wg[:, :, 1] - Wg[:, :, 0]   # [H, 3]
        gy = Wg[:, :, 2] - Wg[:, :, 0]
        folded = [fold(0, gx[:, 0]), fold(1, gx[:, 1]), fold(2, gx[:, 2]),
                  fold(0, gy[:, 0]), fold(1, gy[:, 1]), fold(2, gy[:, 2]),
                  fold(1, ones), fold(0, -ones),
                  fold(2, ones), fold(0, -ones),
                  fold(0, ones)]
        bx = bg[:, 1] - bg[:, 0]
        by = bg[:, 2] - bg[:, 0]
    wq = np.stack([chunked(w, D) for w in folded])             # [NM,128,4,D]

    # per-quad bias columns (value depends on head = 4i + p//32), minus KG
    pp_ = np.arange(128)
    bxq = np.stack([(bx[4 * i + pp_ // 32] - KG).astype(np.float32)[:, None]
                    for i in range(2)])
    byq = np.stack([(by[4 * i + pp_ // 32] - KG).astype(np.float32)[:, None]
                    for i in range(2)])

    common = {"ztf": ztf, "wk": wk, "wq": wq, "bx": bxq, "by": byq}
    return identity_gate, common, zt_full


def kernel(**inputs):
    identity_gate, common, zt_full = _prep_inputs(inputs)

    if identity_gate not in _BUILD_CACHE:
        _BUILD_CACHE[identity_gate] = _build(identity_gate)
    nc = _BUILD_CACHE[identity_gate]

    in_maps = []
    for c in range(N_CORES):
        m = dict(common)
        m["zt"] = np.ascontiguousarray(zt_full[:, :, :, c * GS:(c + 1) * GS])
        in_maps.append(m)

    res = run_bass_kernel_spmd(nc, in_maps, core_ids=list(range(N_CORES)))
    global LAST_RESULT
    LAST_RESULT = res

    u_mean = np.concatenate([res.results[c]["u_mean"] for c in range(N_CORES)],
                            axis=1).astype(np.float32)
    a0m = np.concatenate([res.results[c]["a0_mean"] for c in range(N_CORES)],
                         axis=1).astype(np.float32)
    a1m = np.concatenate([res.results[c]["a1_mean"] for c in range(N_CORES)],
                         axis=1).astype(np.float32)
    A_mean = np.concatenate([res.results[c]["A_mean"] for c in range(N_CORES)],
                            axis=1).astype(np.float32)
    a2m = 1.0 - a0m - a1m
    alpha_mean = np.stack([a0m, a1m, a2m], axis=-1).astype(np.float32)

    H_TF = np.asarray(inputs["H_TF"], np.float32)
    H_G = np.asarray(inputs["H_G"], np.float32)
    return (H_TF, H_G, A_mean.astype(np.float32),
            u_mean.astype(np.float32), alpha_mean)


# revision 25
# speedup vs baseline: 1.0066x; 1.0066x over previous
"""Trainium2 Bass kernel for nn_MultiHeadEFRALayer (sparse_attention).

Strategy: shard the 16384 genes across 8 NeuronCores (2048 each); every core
computes all 8 heads x all 512 TFs for its gene slice.  The 3x3 evidence gate
softmax is reformulated in terms of pairwise score differences so the whole
per-element chain runs in fp16 on VectorE/ScalarE, the per-head scale /
sqrt(dk) and gate-mixing coefficients are folded into the Q-projection
weights on the host, and all means over heads are computed on the TensorE
via selector matmuls (heads are packed into the partition dimension).
Softmax over genes needs the global per-(head,TF) sum of exp(u): a tiny
[128,32] AllReduce mid-kernel provides it; a second pass over the archived
exp(u) tiles (DRAM) produces A_mean.
"""

import sys
sys.path.insert(0, "/opt/trn_rl_repo")

import math
import numpy as np

import concourse.bass as bass
import concourse.mybir as mybir
import concourse.tile as tile
from concourse import bacc
from concourse.bass_utils import run_bass_kernel_spmd

N_CORES = 8
H, DK, D, T, G = 8, 64, 512, 512, 16384
GS = G // N_CORES          # genes per core
KG = 6.0                   # gate-exp shift (overflow guard)
KE = 4.0                   # gene-softmax exp shift
E6 = float(math.exp(KG))
EMK = float(math.exp(-KG))
F32 = mybir.dt.float32
F16 = mybir.dt.float16
AF = mybir.ActivationFunctionType
OP = mybir.AluOpType

_BUILD_CACHE = {}
_ACT_PATCHED = False


def _patch_act_tables():
    """Make Bacc's activation-table chooser pick the combined
    natural_log_exp_and_others set for both Exp and Ln (avoids a ~1.3us
    table reload between every Ln and Exp). We present it a table map with
    Exp/Ln stripped from the exp-only / ln-only sets (dict order, and hence
    act_func_set_id numbering, is unchanged)."""
    global _ACT_PATCHED
    if _ACT_PATCHED:
        return
    try:
        from concourse import bacc as _bacc_mod
        from concourse.hw_specs import get_activation_tables as _orig
    except Exception:
        _ACT_PATCHED = True
        return

    def patched(arch):
        tabs = _orig(arch)
        out = {}
        for name, funcs in tabs.items():
            f = set(funcs)
            if name == "exp_and_others":
                f.discard(AF.Exp)
            if name == "natural_log":
                f.discard(AF.Ln)
            out[name] = f
        return out

    _bacc_mod.get_activation_tables = patched
    _ACT_PATCHED = True


def _unpatch_guard():
    pass
LAST_RESULT = None  # BassKernelResults of the most recent run (for test.py)


def _build(identity_gate: bool):
    """Build the Bacc program. Returns nc."""
    if identity_gate:
        NM, NS = 4, 3
        # bank layout in score PSUM: 0=x, 1=y, 2=s0 ; T12 reads banks 0:2
        # m1 = -Wq_seq/8 is shared by the x and y combos
        combos = {0: [(0, 1), (1, 0)], 1: [(2, 2), (1, 0)], 2: [(3, 0)]}
        XS = 0
    else:
        NM, NS = 10, 5
        # banks: 0=x_l, 1=y_l, 2=x_s, 3=y_s, 4=s0 ; m7 = -Wq_seq/8 shared
        combos = {0: [(0, 0), (1, 1), (2, 2)], 1: [(3, 0), (4, 1), (5, 2)],
                  2: [(6, 1), (7, 0)], 3: [(8, 2), (7, 0)], 4: [(9, 0)]}
        XS = 2
    S0B = NS - 1  # s0 bank index

    _patch_act_tables()
    nc = bacc.Bacc("TRN2", num_devices=N_CORES)

    # ---------------- I/O ----------------
    zt_dr = nc.dram_tensor("zt", (3, 128, 4, GS), F16, kind="ExternalInput")
    ztf_dr = nc.dram_tensor("ztf", (3, 128, 4, T), F16, kind="ExternalInput")
    wk_dr = nc.dram_tensor("wk", (3, 128, 4, D), F16, kind="ExternalInput")
    wq_dr = nc.dram_tensor("wq", (NM, 128, 4, D), F16, kind="ExternalInput")
    bx_dr = nc.dram_tensor("bx", (2, 128, 1), F32, kind="ExternalInput")
    by_dr = nc.dram_tensor("by", (2, 128, 1), F32, kind="ExternalInput")
    um_dr = nc.dram_tensor("u_mean", (T, GS), F16, kind="ExternalOutput")
    a0m_dr = nc.dram_tensor("a0_mean", (T, GS), F16, kind="ExternalOutput")
    a1m_dr = nc.dram_tensor("a1_mean", (T, GS), F16, kind="ExternalOutput")
    Am_dr = nc.dram_tensor("A_mean", (T, GS), F16, kind="ExternalOutput")

    e_arch = nc.dram_tensor("e_arch", (32, 128, GS), F16)
    zc_in = [nc.dram_tensor(f"zc_in{h}", (128, 16), F32) for h in range(2)]
    zc_out = [nc.dram_tensor(f"zc_out{h}", (128, 16), F32, addr_space="Shared")
              for h in range(2)]

    # selector constants: partitions p = h_local*32 + t_local
    pp_, cc_ = np.arange(128)[:, None], np.arange(32)[None, :]
    mask01 = (pp_ % 32 == cc_).astype(np.float16)
    sel8_np = (mask01 / 8.0).astype(np.float16)
    sela0_np = (mask01 * (float(np.exp(-KG)) / 8.0)).astype(np.float16)
    mask_dr = nc.inline_tensor(mask01, "mask01")
    cbias_np = np.tile(np.array([[-KG, EMK, -KE, 0.0]], np.float32), (128, 1))
    cbias_dr = nc.inline_tensor(cbias_np, "cbias")
    sel8_dr = nc.inline_tensor(sel8_np, "sel8")
    sela0_dr = nc.inline_tensor(sela0_np, "sela0")

    with tile.TileContext(nc) as tc:
        with tc.tile_pool(name="persist", bufs=1) as pp, \
             tc.tile_pool(name="psum_s", bufs=(2 if identity_gate else 1), space="PSUM") as ps_s, \
             tc.tile_pool(name="psum_m", bufs=1, space="PSUM") as ps_m:
            cp_ctx = tc.tile_pool(name="chain", bufs=(2 if identity_gate else 1))

            # ---------------- constants / small tiles ----------------
            mask_t = pp.tile([128, 32], F16, tag="mask01")
            sel8_t = pp.tile([128, 32], F16, tag="sel8")
            sela0_t = pp.tile([128, 32], F16, tag="sela0")
            cbias_t = pp.tile([128, 4], F32, tag="cbias")
            nc.sync.dma_start(out=cbias_t[:], in_=cbias_dr[:])
            b_mKG = cbias_t[:, 0:1]
            b_EMK = cbias_t[:, 1:2]
            b_mKE = cbias_t[:, 2:3]
            b_zero = cbias_t[:, 3:4]
            nc.sync.dma_start(out=mask_t[:], in_=mask_dr[:])
            nc.sync.dma_start(out=sel8_t[:], in_=sel8_dr[:])
            nc.sync.dma_start(out=sela0_t[:], in_=sela0_dr[:])
            z_sb = pp.tile([128, 32], F32, tag="z_sb")
            bxq = [pp.tile([128, 1], F32, tag=f"bx{i}", name=f"bx{i}") for i in range(2)]
            byq = [pp.tile([128, 1], F32, tag=f"by{i}", name=f"by{i}") for i in range(2)]
            for i in range(2):
                nc.sync.dma_start(out=bxq[i][:], in_=bx_dr[i])
                nc.sync.dma_start(out=byq[i][:], in_=by_dr[i])

            # ---------------- prologue: load weights, project Q and K ----
            qcT = [pp.tile([128, 4, T], F16, tag=f"qc{m}", name=f"qc{m}") for m in range(NM)]
            kT = [[pp.tile([128, GS], F16, tag=f"kT{e}_{c}", name=f"kT{e}_{c}") for c in range(4)]
                  for e in range(3)]
            PRO_W = 1024 if identity_gate else 512
            NPW = PRO_W // 512

            with tc.tile_pool(name="pro_in", bufs=1) as prop:
                ztf_t = []
                wq_t = []
                for e in range(3):
                    t_ = prop.tile([128, 4, T], F16, tag=f"ztf{e}", name=f"ztf{e}")
                    nc.sync.dma_start(out=t_[:], in_=ztf_dr[e])
                    ztf_t.append(t_)
                for m in range(NM):
                    t_ = prop.tile([128, 4, D], F16, tag="wq", bufs=3, name=f"wq{m}")
                    nc.sync.dma_start(out=t_[:], in_=wq_dr[m])
                    wq_t.append(t_)

                # channel of each folded weight m
                ch_of_m = {m: e for k in combos for (m, e) in combos[k]}

                # Q projections: qcT[m][:, ch, :] = sum_c wq_m[c]^T-block @ ztf
                for m in range(NM):
                    e = ch_of_m[m]
                    for blk in range(4 // NPW):
                        qps = ps_m.tile([128, NPW, 512], F32, tag="mean", name="qps")
                        for cc in range(NPW):
                            ch = blk * NPW + cc
                            for c in range(4):
                                nc.tensor.matmul(
                                    out=qps[:, cc, :],
                                    lhsT=wq_t[m][:, c, ch * 128:(ch + 1) * 128],
                                    rhs=ztf_t[e][:, c, :],
                                    start=(c == 0), stop=(c == 3))
                        if m % 2 == 0:
                            nc.scalar.copy(out=qcT[m][:, blk * NPW:(blk + 1) * NPW, :],
                                           in_=qps[:])
                        else:
                            nc.vector.tensor_copy(out=qcT[m][:, blk * NPW:(blk + 1) * NPW, :],
                                                  in_=qps[:])

                wk_t = []
                zt_t = []
                for e in range(3):
                    t_ = prop.tile([128, 4, D], F16, tag=f"wk{e}", name=f"wk{e}")
                    nc.sync.dma_start(out=t_[:], in_=wk_dr[e])
                    wk_t.append(t_)
                    t_ = prop.tile([128, 4, GS], F16, tag=f"zt{e}", name=f"zt{e}")
                    nc.sync.dma_start(out=t_[:], in_=zt_dr[e])
                    zt_t.append(t_)

                # K projections: kT[e][ch] = sum_c wk[e][c]^T-block @ zt[e][c]
                # ch-major so the first supertiles' inputs finish first
                for ch in range(4):
                    for e in range(3):
                        for blk in range(GS // 512 // NPW):
                            kps = ps_m.tile([128, NPW, 512], F32, tag="mean", name="kps")
                            for gg in range(NPW):
                                g0 = blk * PRO_W + gg * 512
                                for c in range(4):
                                    nc.tensor.matmul(
                                        out=kps[:, gg, :],
                                        lhsT=wk_t[e][:, c, ch * 128:(ch + 1) * 128],
                                        rhs=zt_t[e][:, c, g0:g0 + 512],
                                        start=(c == 0), stop=(c == 3))
                            if (ch + blk) % 2 == 0:
                                nc.scalar.copy(
                                    out=kT[e][ch][:, blk * PRO_W:(blk + 1) * PRO_W],
                                    in_=kps[:])
                            else:
                                nc.vector.tensor_copy(
                                    out=kT[e][ch][:, blk * PRO_W:(blk + 1) * PRO_W],
                                    in_=kps[:])

            cp = cp_ctx.__enter__()
            # ---------------- main loop ----------------
            zg = pp.tile([128, 32], F32, tag="zg")
            rz = pp.tile([128, 32], F32, tag="rz")

            def z_allreduce(hf):
                cs = slice(hf * 16, (hf + 1) * 16)
                nc.sync.dma_start(out=zc_in[hf][:], in_=z_sb[:, cs])
                nc.gpsimd.collective_compute(
                    "AllReduce", OP.add,
                    replica_groups=[[r for r in range(N_CORES)]],
                    ins=[zc_in[hf][:]], outs=[zc_out[hf][:]])
                nc.sync.dma_start(out=zg[:, cs], in_=zc_out[hf][:])
                nc.vector.reciprocal(out=rz[:, cs], in_=zg[:, cs])
                # rz := 2^14 / (8 * Z)
                nc.vector.tensor_scalar_mul(rz[:, cs], rz[:, cs],
                                            float(2.0 ** 14 / 8.0))

            ut_t = [None, None]
            a0_t = [None, None]
            v1_t = [None, None]
            for j in range(16):            # t-block of 32 TFs
                for i in range(2):         # head quad (heads 4i..4i+3)
                    st = i * 16 + j
                    zcol = j * 2 + i
                    e_cat = cp.tile([128, 2, GS], F16, tag="ecat", name=f"ecat{i}")
                    c0 = cp.tile([128, GS], F16, tag="c0", name=f"c0{i}")
                    t12 = cp.tile([128, 2, GS], F16, tag="t12", name=f"t12{i}")
                    for q in range(4):
                        qs = slice(q * 512, (q + 1) * 512)
                        sxy = ps_s.tile([128, NS, 512], F32, tag="s", name="sxy")
                        # ---- score matmuls ----
                        # Emission interleaves banks so that one bank's next
                        # col-group start (bank-bit clear) never races the
                        # previous group's accumulate drain.
                        max_p = max(len(v) for v in combos.values())
                        for pi in range(max_p):
                            for k in range(NS):
                                plist = combos[k]
                                if pi >= len(plist):
                                    continue
                                m, e = plist[pi]
                                for hm in range(4):
                                    head = 4 * i + hm
                                    ch = head // 2
                                    po = (head % 2) * 64
                                    nc.tensor.matmul(
                                        out=sxy[hm * 32:(hm + 1) * 32, k, :],
                                        lhsT=qcT[m][po:po + 64, ch,
                                                    j * 32:(j + 1) * 32],
                                        rhs=kT[e][ch][po:po + 64, qs],
                                        start=(pi == 0),
                                        stop=(pi == len(plist) - 1),
                                        tile_position=(po, hm * 32),
                                        skip_group_check=True)
                        # ---- chain entry (per q) ----
                        if identity_gate:
                            nc.scalar.activation(out=e_cat[:, :, qs],
                                                 in_=sxy[:, 0:2, :],
                                                 func=AF.Exp, bias=b_mKG)
                        else:
                            nc.scalar.activation(out=e_cat[:, 0, qs],
                                                 in_=sxy[:, 0, :],
                                                 func=AF.Exp, bias=bxq[i][:])
                            nc.scalar.activation(out=e_cat[:, 1, qs],
                                                 in_=sxy[:, 1, :],
                                                 func=AF.Exp, bias=byq[i][:])
                        nc.vector.tensor_tensor(out=t12[:, :, qs],
                                                in0=sxy[:, XS:XS + 2, :],
                                                in1=e_cat[:, :, qs], op=OP.mult)
                        if q < 2:
                            nc.scalar.copy(out=c0[:, qs], in_=sxy[:, S0B, :])
                        else:
                            nc.vector.tensor_copy(out=c0[:, qs],
                                                  in_=sxy[:, S0B, :])
                    # ---- chain mid (full 2048) ----
                    tsum = cp.tile([128, GS], F16, tag="tsum", name=f"tsum{i}")
                    nc.vector.tensor_add(out=tsum[:], in0=e_cat[:, 0, :],
                                         in1=e_cat[:, 1, :])
                    Lt = cp.tile([128, GS], F16, tag="L", name=f"L{i}")
                    nc.scalar.activation(out=Lt[:], in_=tsum[:], func=AF.Ln,
                                         bias=b_EMK)
                    a0 = cp.tile([128, GS], F16, tag=f"a0{i}", name=f"a0{i}", bufs=1)
                    nc.scalar.activation(out=a0[:], in_=Lt[:], func=AF.Exp,
                                         scale=-1.0, bias=b_zero)
                    Pp = cp.tile([128, GS], F16, tag="Pp", name=f"Pp{i}")
                    nc.vector.tensor_add(out=Pp[:], in0=t12[:, 0, :],
                                         in1=t12[:, 1, :])
                    Wt = cp.tile([128, GS], F16, tag="Wt", name=f"Wt{i}", bufs=1)
                    nc.vector.tensor_mul(out=Wt[:], in0=Pp[:], in1=a0[:])
                    ut = cp.tile([128, GS], F16, tag=f"ut{i}", name=f"ut{i}", bufs=1)
                    nc.vector.tensor_add(out=ut[:], in0=Wt[:], in1=c0[:])
                    Et = cp.tile([128, GS], F16, tag="Et", name=f"Et{i}")
                    nc.scalar.activation(out=Et[:], in_=ut[:], func=AF.Exp,
                                         bias=b_mKE, accum_out=z_sb[:, zcol:zcol + 1])
                    v1 = cp.tile([128, GS], F16, tag=f"v1{i}", name=f"v1{i}", bufs=1)
                    nc.vector.tensor_mul(out=v1[:], in0=e_cat[:, 0, :],
                                         in1=a0[:])
                    nc.sync.dma_start(out=e_arch[st], in_=Et[:])
                    ut_t[i], a0_t[i], v1_t[i] = ut, a0, v1

                # ---- head-mean matmuls (one PSUM tile per quad; the two
                # quads are combined by the fused DVE add so the shared mean
                # slot is released quickly between chains) ----
                stgF = cp.tile([128, 2, 2, 512], F16, tag="mstageF",
                               name="stgF")
                for half in range(2):
                    stg0 = cp.tile([128, 2, 512], F32, tag="mstage0", name="stg0")
                    for i in range(2):
                        mps = ps_m.tile([128, 2, 512], F32, tag="mean",
                                        name="mps")
                        for gg in range(2):
                            g0 = half * 1024 + gg * 512
                            for bp, rhs_t, sel_t in (
                                    (0, ut_t[i], sel8_t),
                                    (32, a0_t[i], sela0_t),
                                    (64, v1_t[i], sel8_t)):
                                nc.tensor.matmul(
                                    out=mps[bp:bp + 32, gg, :],
                                    lhsT=sel_t[:],
                                    rhs=rhs_t[:, g0:g0 + 512],
                                    start=True, stop=True,
                                    tile_position=(0, bp),
                                    skip_group_check=True)
                        if i == 0:
                            nc.scalar.copy(out=stg0[0:96, :, :],
                                           in_=mps[0:96, :, :])
                        else:
                            nc.vector.scalar_tensor_tensor(
                                out=stgF[0:96, half, :, :], in0=mps[0:96, :, :],
                                scalar=1.0, in1=stg0[0:96, :, :],
                                op0=OP.mult, op1=OP.add)
                rs = slice(j * 32, (j + 1) * 32)
                nc.sync.dma_start(out=um_dr[rs, :], in_=stgF[0:32])
                nc.sync.dma_start(out=a0m_dr[rs, :], in_=stgF[32:64])
                nc.sync.dma_start(out=a1m_dr[rs, :], in_=stgF[64:96])
                if j == 7:
                    z_allreduce(0)
                elif j == 15:
                    z_allreduce(1)

            # ---------------- pass 2 (A_mean) ----------------
            for j in range(16):
                Er = [None, None]
                selr = [None, None]
                for i in range(2):
                    st = i * 16 + j
                    zcol = j * 2 + i
                    Er[i] = cp.tile([128, GS], F16, tag=f"Ere{i}", name=f"Ere{i}", bufs=1)
                    nc.sync.dma_start(out=Er[i][:], in_=e_arch[st])
                    selr[i] = cp.tile([128, 32], F16, tag=f"selr{i}", name=f"selr{i}")
                    nc.vector.tensor_scalar(out=selr[i][:], in0=mask_t[:],
                                            scalar1=rz[:, zcol:zcol + 1],
                                            scalar2=None, op0=OP.mult)
                astgF = cp.tile([128, 2, 2, 512], F16, tag="astageF",
                                name="astgF", bufs=1)
                for half in range(2):
                    astg0 = cp.tile([128, 2, 512], F32, tag="astage0",
                                    name="astg0", bufs=1)
                    for i in range(2):
                        aps = ps_m.tile([128, 2, 512], F32, tag="mean",
                                        name="aps")
                        for gg in range(2):
                            g0 = half * 1024 + gg * 512
                            nc.tensor.matmul(
                                out=aps[0:32, gg, :], lhsT=selr[i][:],
                                rhs=Er[i][:, g0:g0 + 512],
                                start=True, stop=True,
                                tile_position=(0, 0),
                                skip_group_check=True)
                        if i == 0:
                            nc.scalar.mul(out=astg0[0:32, :, :],
                                          in_=aps[0:32, :, :],
                                          mul=float(2.0 ** -14))
                        else:
                            nc.vector.scalar_tensor_tensor(
                                out=astgF[0:32, half, :, :], in0=aps[0:32, :, :],
                                scalar=float(2.0 ** -14), in1=astg0[0:32, :, :],
                                op0=OP.mult, op1=OP.add)
                rs = slice(j * 32, (j + 1) * 32)
                nc.sync.dma_start(out=Am_dr[rs, :], in_=astgF[0:32])
            cp_ctx.__exit__(None, None, None)

    nc.compile()
    return nc


def _prep_inputs(inputs):
    """Host-side prep: gather, transpose, fold, fp16 pack. Returns
    (identity_gate, common in_map pieces, per-core zt slices)."""
    f16 = np.float16
    tf_idx = np.asarray(inputs["tf_idx"])
    Wg = np.asarray(inputs["gate_weight"], dtype=np.float32)
    bg = np.asarray(inputs["gate_bias"], dtype=np.float32)
    zs = [np.asarray(inputs["z_seq"], np.float32),
          np.asarray(inputs["z_exp"], np.float32),
          np.asarray(inputs["z_txt"], np.float32)]
    Wqs = [np.asarray(inputs["Wq_seq"], np.float32),
           np.asarray(inputs["Wq_exp"], np.float32),
           np.asarray(inputs["Wq_txt"], np.float32)]
    Wks = [np.asarray(inputs["Wk_seq"], np.float32),
           np.asarray(inputs["Wk_exp"], np.float32),
           np.asarray(inputs["Wk_txt"], np.float32)]

    eye = np.tile(np.eye(3, dtype=np.float32)[None], (H, 1, 1))
    identity_gate = bool(np.allclose(Wg, eye) and np.allclose(bg, 0.0))

    def chunked(mat, width):
        # [D, width] -> [128, 4, width] with [p, c, :] = mat[c*128+p, :]
        return np.ascontiguousarray(
            mat.reshape(4, 128, width).transpose(1, 0, 2)).astype(f16)

    # z^T  [D, G] -> zt[e]: [128, 4, G] (sliced per core later)
    zt_full = np.stack([chunked(z.T, G) for z in zs])          # [3,128,4,G]
    ztf = np.stack([chunked(z[tf_idx].T, T) for z in zs])      # [3,128,4,T]
    wk = np.stack([chunked(w, D) for w in Wks])                # [3,128,4,D]

    def fold(e, coef_per_head):
        # Wq_e scaled per head by coef/8 (score scale 1/sqrt(dk)=1/8)
        return Wqs[e] * np.repeat(coef_per_head / 8.0, DK)[None, :]

    ones = np.ones(H, np.float32)
    if identity_gate:
        folded = [fold(1, ones), fold(0, -ones),
                  fold(2, ones),
                  fold(0, ones)]
        bx = np.zeros(H, np.float32)
        by = np.zeros(H, np.float32)
    else:
        gx = Wg[:, :, 1] - Wg[:, :, 0]   # [H, 3]
        gy = Wg[:, :, 2] - Wg[:, :, 0]
        folded = [fold(0, gx[:, 0]), fold(1, gx[:, 1]), fold(2, gx[:, 2]),
                  fold(0, gy[:, 0]), fold(1, gy[:, 1]), fold(2, gy[:, 2]),
                  fold(1, ones), fold(0, -ones),
                  fold(2, ones),
                  fold(0, ones)]
        bx = bg[:, 1] - bg[:, 0]
        by = bg[:, 2] - bg[:, 0]
    wq = np.stack([chunked(w, D) for w in folded])             # [NM,128,4,D]

    # per-quad bias columns (value depends on head = 4i + p//32), minus KG
    pp_ = np.arange(128)
    bxq = np.stack([(bx[4 * i + pp_ // 32] - KG).astype(np.float32)[:, None]
                    for i in range(2)])
    byq = np.stack([(by[4 * i + pp_ // 32] - KG).astype(np.float32)[:, None]
                    for i in range(2)])

    common = {"ztf": ztf, "wk": wk, "wq": wq, "bx": bxq, "by": byq}
    return identity_gate, common, zt_full


def kernel(**inputs):
    identity_gate, common, zt_full = _prep_inputs(inputs)

    if identity_gate not in _BUILD_CACHE:
        _BUILD_CACHE[identity_gate] = _build(identity_gate)
    nc = _BUILD_CACHE[identity_gate]

    in_maps = []
    for c in range(N_CORES):
        m = dict(common)
        m["zt"] = np.ascontiguousarray(zt_full[:, :, :, c * GS:(c + 1) * GS])
        in_maps.append(m)

    res = run_bass_kernel_spmd(nc, in_maps, core_ids=list(range(N_CORES)))
    global LAST_RESULT
    LAST_RESULT = res

    u_mean = np.concatenate([res.results[c]["u_mean"] for c in range(N_CORES)],
                            axis=1).astype(np.float32)
    a0m = np.concatenate([res.results[c]["a0_mean"] for c in range(N_CORES)],
                         axis=1).astype(np.float32)
    a1m = np.concatenate([res.results[c]["a1_mean"] for c in range(N_CORES)],
                         axis=1).astype(np.float32)
    A_mean = np.concatenate([res.results[c]["A_mean"] for c in range(N_CORES)],
                            axis=1).astype(np.float32)
    a2m = 1.0 - a0m - a1m
    alpha_mean = np.stack([a0m, a1m, a2m], axis=-1).astype(np.float32)

    H_TF = np.asarray(inputs["H_TF"], np.float32)
    H_G = np.asarray(inputs["H_G"], np.float32)
    return (H_TF, H_G, A_mean.astype(np.float32),
            u_mean.astype(np.float32), alpha_mean)
